# revision 4
# baseline (speedup 1.0000x reference)
"""Trainium2 Bass kernel for a GQA attention block (dense_transformer).

Reference computation (fp32):
    q = h @ Wq.T; k = h @ Wk.T; v = h @ Wv.T        (h: [2048, 4096])
    q, k = rope(q), rope(k)
    attn = softmax_causal(q k^T / sqrt(128)) v       (32 q-heads, 8 kv-heads)
    out = attn @ Wo.T
Sharding: tensor-parallel over heads. Core c owns q-heads 4c..4c+3 and
kv-head c; it computes a full [2048, 4096] partial of the output
projection and the host sums the 8 partials.

Fused-pipeline design (vs the 2-phase baseline at ~412us):
- Single software pipeline over 512-row sequence strips: attention on
  strip j runs with projections+rope of strip j+1 and o_proj of strip
  j-1 interleaved as PE filler ("pump"), so the tensor engine never
  idles on the scalar engine's exp chains and never sees a phase
  boundary.
- Scores/att matmuls are merged per head-PAIR (one matmul covers both
  heads via a strided 3D access pattern on qTp [128, strip, head, 512]),
  halving instruction/LDWEIGHTS count in attention.
- rope's rotate-half runs as two SBUF->SBUF partition-swap DMAs (sign
  folded into a host-negated sinT) instead of PE rot-matmuls.
- Softmax denominator: one all-ones [128,128] stationary matmul
  partition-sums AND broadcasts Sum(exp) to all partitions in one shot;
  1/x = exp(-ln x) on ScalarE; DVE multiplies into aT.
- PSUM (8 banks x 2KB): att accum [128,2,512] (2) + scores [128,2,512]
  (2) + projection pass q0,q1,k | q2,q3,v [128,3,512] (3) + spare
  [128,512] (1) for o_proj groups / v-transposes. Projections run two
  passes per strip (hT re-DMA'd per pass; DMA has headroom).
- o_proj emits as 32 single-m groups per strip (4 matmuls sharing the
  psum spare), output DMA on the gpsimd queue.
"""

import sys

sys.path.insert(0, "/opt/trn_rl_repo")

import numpy as np
import ml_dtypes

import concourse.bass as bass
import concourse.tile as tile
from concourse import mybir, bass_isa
from concourse.bass_utils import run_bass_kernel_spmd
from bass_rust import ScopedClock, VectorClock

HIDDEN = 4096
N_HEADS = 32
N_KV = 8
HEAD_DIM = 128
S = 2048
ROPE_BASE = 10000.0
N_CORES = 8
QH = N_HEADS // N_CORES  # q heads per core = 4
SCALE = HEAD_DIM**-0.5

F32 = mybir.dt.float32
F32R = mybir.dt.float32r
BF = mybir.dt.bfloat16
AF = mybir.ActivationFunctionType
ALU = mybir.AluOpType

KT = HIDDEN // 128  # 32 contraction tiles for the projections
NSTRIP = S // 512  # 4 sequence strips of 512
KC = 4  # hidden k-tiles per weight/hT chunk

_MAX_CTRL_WAITS = 2


class _SplitDrainTileContext(tile.TileContext):
    """Walrus in this env caps embedded sync waits per instruction (2 for
    CTRL/LW struct types). Tile can attach more. The tail drain is handled
    here (waits moved onto SP nops before the drain); every other
    instruction is handled by _split_excess_waits() after emission."""

    def _drain_and_barrier(self, tick_clock, wait_clock):
        gc = tick_clock.global_clock
        for scope, v in ScopedClock({None: gc}).items():
            n = len(v)
            for proc in range(n):
                tick = v[proc]
                if tick <= 0:
                    continue
                partial = ScopedClock(
                    {scope: VectorClock([tick if i == proc else 0 for i in range(n)])}
                )
                nop = self.nc.sync.nop(nofuse=True, hint="drain_split")
                wait_clock.add_sem_waits(nop.ins, partial)

        drain_inst = self.nc.sync.drain()
        wait_clock.add_sem_waits(
            drain_inst.ins, ScopedClock({None: tick_clock.global_clock})
        )
        si = drain_inst.ins.sync_info
        if si is not None and len(si.on_wait) > _MAX_CTRL_WAITS:
            drain_inst.ins.sync_info = mybir.SyncInfo(
                on_wait=[], on_update=list(si.on_update)
            )

        self.nc.all_engine_barrier()
        assert self.sems is not None
        popped = self.nc._tile_sem_poison_stack.pop()
        assert popped is self._sem_poison
        self.nc.clear_and_free_semaphores(list(self.sems.allocated().values()))
        self.nc.all_engine_barrier()


def _split_excess_waits(nc, cap=1):
    """Rebuild basic blocks so no instruction carries more than `cap` sem
    waits; excess waits move onto same-engine NoOps placed just before the
    instruction (same AND semantics, engine blocks at each nop in turn)."""
    import bass_rust as _br

    nsplit = 0
    for fn in nc.m.functions:
        new_blocks = []
        rebuilt_any = False
        for bb in fn.blocks:
            insts = bb.instructions
            need = any(
                (inst.sync_info is not None and len(inst.sync_info.on_wait) > cap)
                for inst in insts
            )
            if not need:
                new_blocks.append(bb)
                continue
            rebuilt_any = True
            out = []
            for inst in insts:
                si = inst.sync_info
                if si is not None and len(si.on_wait) > cap:
                    waits = list(si.on_wait)
                    extra, keep = waits[:-cap], waits[-cap:]
                    for i in range(0, len(extra), cap):
                        nop = mybir.InstNoOp(
                            name=f"{inst.name}.w{i}", ins=[], outs=[]
                        )
                        nop.engine = inst.engine
                        nop.sync_info = mybir.SyncInfo(
                            on_wait=extra[i : i + cap], on_update=[]
                        )
                        out.append(nop)
                        nsplit += 1
                    inst.sync_info = mybir.SyncInfo(
                        on_wait=keep, on_update=list(si.on_update)
                    )
                out.append(inst)
            nb = _br.BasicBlock(name=bb.name, instructions=out)
            nb.IsExit = bb.IsExit
            nb.IsLoopEntry = bb.IsLoopEntry
            nb.IsPredicated = bb.IsPredicated
            new_blocks.append(nb)
        if rebuilt_any:
            fn.blocks = new_blocks
    return nsplit


def _emit(nc):
    hT = nc.declare_dram_parameter("hT", [HIDDEN, S], BF, isOutput=False)
    wqT = nc.declare_dram_parameter("wqT", [HIDDEN, QH * HEAD_DIM], BF, isOutput=False)
    wkT = nc.declare_dram_parameter("wkT", [HIDDEN, HEAD_DIM], BF, isOutput=False)
    wvT = nc.declare_dram_parameter("wvT", [HIDDEN, HEAD_DIM], BF, isOutput=False)
    woT = nc.declare_dram_parameter("woT", [QH * HEAD_DIM, HIDDEN], BF, isOutput=False)
    cosT = nc.declare_dram_parameter("cosT", [128, S], BF, isOutput=False)
    # sinT is host-negated on rows 0:64, so rotate-half is a pure
    # partition-swap DMA (no sign flip on device)
    sinT = nc.declare_dram_parameter("sinT", [128, S], BF, isOutput=False)
    ident = nc.declare_dram_parameter("ident", [128, 128], BF, isOutput=False)
    masksd = nc.declare_dram_parameter("masks", [128, 2 * 128], BF, isOutput=False)
    onesd = nc.declare_dram_parameter("ones", [128, 128], BF, isOutput=False)
    # bf16 output: the host upcasts and sums the 8 partials
    out = nc.declare_dram_parameter("o", [S, HIDDEN], BF, isOutput=True)

    hT3 = hT[:].rearrange("(k p) s -> p k s", p=128)
    wq3 = wqT[:].rearrange("(k p) m -> p k m", p=128)
    wk3 = wkT[:].rearrange("(k p) m -> p k m", p=128)
    wv3 = wvT[:].rearrange("(k p) m -> p k m", p=128)
    wo3 = woT[:].rearrange("(k p) m -> p k m", p=128)

    with _SplitDrainTileContext(nc) as tc:
        with (
            tc.tile_pool(name="consts", bufs=1) as pc,
            tc.tile_pool(name="persist", bufs=1) as pp,
            tc.tile_pool(name="pw", bufs=1) as pw,
            tc.tile_pool(name="ph", bufs=3) as ph,
            tc.tile_pool(name="pstage", bufs=2) as ps,
            tc.tile_pool(name="pex", bufs=1) as px,
            tc.tile_pool(name="po", bufs=1) as po,
            tc.tile_pool(name="psum", bufs=1, space="PSUM") as pq,
        ):
            cos_sb = pc.tile([128, S], BF, tag="cos")
            sin_sb = pc.tile([128, S], BF, tag="sin")
            id_sb = pc.tile([128, 128], BF, tag="id")
            mask_sb = pc.tile([128, 2, 128], BF, tag="mask")
            ones_sb = pc.tile([128, 128], BF, tag="ones")

            # qTp[pr]: strip-major, head-pair-minor layout so one score/att
            # matmul covers both heads of a pair via a 3D AP
            qTp = [
                pp.tile([128, NSTRIP, 2, 512], BF, tag=f"qTp{pr}", name=f"qTp{pr}")
                for pr in range(2)
            ]
            kT = pp.tile([128, S], BF, tag="kT")
            vsb = pp.tile([128, S], BF, tag="v")  # [sk-part, 16 tiles x 128 d]
            aT = [pp.tile([128, S], BF, tag=f"aT{h}", name=f"aT{h}") for h in range(QH)]
            wo_sb = pp.tile([128, QH, HIDDEN], BF, tag="wo")

            # ---- weight chunks (streamed during strip 0) ----
            CS = [1, 1, 2] + [4] * 7  # k-tiles per chunk, sums to KT=32
            CO = [sum(CS[:i]) for i in range(len(CS))]
            wq_c = [
                pw.tile([128, csz, QH * 128], BF, tag=f"wq{ci}", name=f"wq{ci}")
                for ci, csz in enumerate(CS)
            ]
            wk_c = [
                pw.tile([128, csz, 128], BF, tag=f"wk{ci}", name=f"wk{ci}")
                for ci, csz in enumerate(CS)
            ]
            wv_c = [
                pw.tile([128, csz, 128], BF, tag=f"wv{ci}", name=f"wv{ci}")
                for ci, csz in enumerate(CS)
            ]

            def wchunk(kt_i):
                # (weight chunk index, index within chunk) for global k-tile
                for c in range(len(CS) - 1, -1, -1):
                    if kt_i >= CO[c]:
                        return c, kt_i - CO[c]
                raise AssertionError

            # ---------------- helpers ----------------
            osb_alt = [0]

            def rope_head(dst, raw, rot, sl):
                # dst = raw*cos + rot*sin'  (sin' sign-folded)
                nc.gpsimd.tensor_tensor(dst, raw, cos_sb[:, sl], ALU.mult)
                tmp = ps.tile([128, 512], BF, tag="tmp", name="tmp", bufs=4)
                nc.vector.tensor_tensor(tmp[:], rot, sin_sb[:, sl], ALU.mult)
                nc.vector.tensor_tensor(dst, dst, tmp[:], ALU.add)

            def rope_A(j2, ps_q, ps_k):
                # pair-0 heads (0,1) + k for strip j2. ps_q: [128,2,512] psum
                # view, ps_k: [128,512] view.
                sl = slice(j2 * 512, (j2 + 1) * 512)
                qraw = ps.tile([128, 2, 512], BF, tag="qraw", name=f"qrawA{j2}", bufs=2)
                nc.scalar.copy(qraw[:], ps_q)
                kraw = ps.tile([128, 512], BF, tag="kraw", name=f"kraw{j2}", bufs=2)
                nc.vector.tensor_copy(kraw[:], ps_k)
                qrot = ps.tile([128, 2, 512], BF, tag="qrot", name=f"qrotA{j2}", bufs=2)
                nc.gpsimd.dma_start(qrot[0:64, :, :], qraw[64:128, :, :])
                nc.gpsimd.dma_start(qrot[64:128, :, :], qraw[0:64, :, :])
                krot = ps.tile([128, 512], BF, tag="krot", name=f"krot{j2}", bufs=2)
                nc.gpsimd.dma_start(krot[0:64, :], kraw[64:128, :])
                nc.gpsimd.dma_start(krot[64:128, :], kraw[0:64, :])
                for hh in range(2):
                    rope_head(
                        qTp[0][:, j2, hh, :], qraw[:, hh, :], qrot[:, hh, :], sl
                    )
                rope_head(kT[:, sl], kraw[:], krot[:], sl)

            def rope_B(j2, ps_q, ps_v):
                # pair-1 heads (2,3) + v transpose for strip j2
                sl = slice(j2 * 512, (j2 + 1) * 512)
                qraw = ps.tile([128, 2, 512], BF, tag="qraw", name=f"qrawB{j2}", bufs=2)
                nc.scalar.copy(qraw[:], ps_q)
                vraw = ps.tile([128, 512], BF, tag="vraw", name=f"vraw{j2}", bufs=2)
                nc.vector.tensor_copy(vraw[:], ps_v)
                qrot = ps.tile([128, 2, 512], BF, tag="qrot", name=f"qrotB{j2}", bufs=2)
                nc.gpsimd.dma_start(qrot[0:64, :, :], qraw[64:128, :, :])
                nc.gpsimd.dma_start(qrot[64:128, :, :], qraw[0:64, :, :])
                for hh in range(2):
                    rope_head(
                        qTp[1][:, j2, hh, :], qraw[:, hh, :], qrot[:, hh, :], sl
                    )
                for t2 in range(4):
                    tr = pq.tile([128, 128], BF, tag="sp", name=f"tr{j2}_{t2}")
                    nc.tensor.transpose(
                        tr[:], vraw[:, t2 * 128 : (t2 + 1) * 128], id_sb[:]
                    )
                    it = j2 * 4 + t2
                    nc.vector.tensor_copy(vsb[:, it * 128 : (it + 1) * 128], tr[:])

            # ---------------- o_proj units ----------------
            def o_unit(jj, stt, mt, tag):
                stg = jj * 4 + stt
                ssl = slice(stg * 128, (stg + 1) * 128)
                o_ps = pq.tile([128, 512], F32, tag=tag, name=f"o{stg}_{mt}")
                for k4 in range(QH):
                    nc.tensor.matmul(
                        o_ps[:],
                        aT[k4][:, ssl],
                        wo_sb[:, k4, mt * 512 : (mt + 1) * 512],
                        start=(k4 == 0),
                        stop=(k4 == QH - 1),
                    )
                osb = po.tile([128, 512], BF, tag="osb", name=f"osb{stg}_{mt}", bufs=6)
                if osb_alt[0] % 2 == 0:
                    nc.vector.tensor_copy(osb[:], o_ps[:])
                else:
                    nc.scalar.copy(osb[:], o_ps[:])
                osb_alt[0] += 1
                nc.gpsimd.dma_start(out[ssl, mt * 512 : (mt + 1) * 512], osb[:])

            # ---------------- pump (PE filler queues) ----------------
            # proj units must drain within their segment (in order); o units
            # may carry over. Each unit: (cycles, closure).
            proj_q = []
            o_q = []
            pump_state = {"per_slot": 0, "credit": 0, "o_tags": ["sp"]}
            o_rr = [0]

            def pump_one():
                # prefer draining proportionally: alternate, proj first
                if proj_q and o_q:
                    # keep ratio: pick the queue with more remaining cycles
                    pc_ = sum(c for c, _ in proj_q)
                    oc_ = sum(c for c, _ in o_q)
                    q = proj_q if pc_ >= oc_ else o_q
                elif proj_q:
                    q = proj_q
                elif o_q:
                    q = o_q
                else:
                    return 0
                cyc, fn = q.pop(0)
                fn()
                return cyc

            def pump():
                budget = pump_state["per_slot"] + pump_state["credit"]
                spent = 0
                while spent < budget:
                    c = pump_one()
                    if c == 0:
                        break
                    spent += c
                pump_state["credit"] = budget - spent if spent < budget else 0

            def o_tag_next():
                tags = pump_state["o_tags"]
                t = tags[o_rr[0] % len(tags)]
                o_rr[0] += 1
                return t

            # ---------------- projection pass units for strips 1-3 --------
            def build_proj_units(j2):
                """Two passes over k for strip j2: pass A accumulates
                (q0,q1,k), pass B (q2,q3,v); hT chunks re-DMA'd per pass.
                Returns ordered unit list (incl. rope/vtrans closures)."""
                sl = slice(j2 * 512, (j2 + 1) * 512)
                units = []
                psA = pq.tile([128, 3, 512], F32, tag="proj", name=f"psA{j2}")
                psB = pq.tile([128, 3, 512], F32, tag="proj", name=f"psB{j2}")

                def mk_pass(pspass, heads, kv_w, label):
                    # units of 2 k-tiles; chunk DMA issued at its first unit,
                    # one chunk of lookahead
                    ht_tiles = {}

                    def get_ht(cc):
                        if cc not in ht_tiles and cc * KC < KT:
                            t = ph.tile(
                                [128, KC, 512], BF, tag="ht4",
                                name=f"ht{label}{j2}_{cc}",
                            )
                            kcs = slice(cc * KC, (cc + 1) * KC)
                            nc.sync.dma_start(t[:], hT3[:, kcs, sl])
                            ht_tiles[cc] = t
                        return ht_tiles.get(cc)

                    def unit(u):
                        # u covers k-tiles 2u, 2u+1
                        cc = (2 * u) // KC
                        ht = get_ht(cc)
                        if u % 2 == 0:
                            get_ht(cc + 1)  # lookahead
                        for kk2 in range(2):
                            kt_i = 2 * u + kk2
                            st = kt_i == 0
                            sp_ = kt_i == KT - 1
                            rhs = ht[:, kt_i - cc * KC, :]
                            ci, kof = wchunk(kt_i)
                            for sidx, h in enumerate(heads):
                                nc.tensor.matmul(
                                    pspass[:, sidx, :],
                                    wq_c[ci][:, kof, h * 128 : (h + 1) * 128],
                                    rhs,
                                    start=st,
                                    stop=sp_,
                                )
                            nc.tensor.matmul(
                                pspass[:, 2, :],
                                kv_w[ci][:, kof, :],
                                rhs,
                                start=st,
                                stop=sp_,
                            )
                        return None

                    return [
                        (2 * 3 * 512, (lambda u=u: unit(u))) for u in range(KT // 2)
                    ]

                units += mk_pass(psA, (0, 1), wk_c, "a")
                units.append((0, lambda: rope_A(j2, psA[:, 0:2, :], psA[:, 2, :])))
                units += mk_pass(psB, (2, 3), wv_c, "b")
                units.append((512, lambda: rope_B(j2, psB[:, 0:2, :], psB[:, 2, :])))
                return units

            # ---------------- attention ----------------
            def attn_pair(j, pr):
                jsl = slice(j * 512, (j + 1) * 512)
                ni = 4 * j + 4
                att_t = pq.tile([128, 2, 512], F32, tag="att", name=f"att{j}_{pr}")
                acc = px.tile(
                    [128, 2, 512], BF, tag="acc", name=f"acc{j}_{pr}", bufs=3
                )
                pend = None
                for i in range(ni):
                    # filler FIRST: the PE runs its queue in order, so
                    # filler must precede the dependent score matmul
                    pump()
                    r = i - 4 * j
                    c0 = 128 * r if r > 0 else 0
                    sc_t = pq.tile(
                        [128, 2, 512], F32, tag="sc", name=f"sc{j}_{pr}_{i}"
                    )
                    # the ISA caps a matmul's moving operand at 512 elements
                    # (s3d3_mm_num_elements), so one matmul per head
                    for hh in range(2):
                        nc.tensor.matmul(
                            sc_t[:, hh, c0:],
                            kT[:, i * 128 : (i + 1) * 128],
                            qTp[pr][:, j, hh, c0:],
                            start=True,
                            stop=True,
                        )
                    if i == 0:
                        ex = acc
                    else:
                        ex = px.tile(
                            [128, 2, 512], BF, tag="ex", name=f"ex{j}_{pr}_{i}",
                            bufs=6,
                        )
                    nc.scalar.activation(
                        ex[:, :, c0:], sc_t[:, :, c0:], AF.Exp, scale=float(SCALE)
                    )
                    if r >= 0:
                        nc.vector.tensor_tensor(
                            ex[:, :, c0 : c0 + 128],
                            ex[:, :, c0 : c0 + 128],
                            mask_sb[:],
                            ALU.mult,
                        )
                    if pend is not None:
                        flush(att_t, acc, ni, pend)
                    pend = (i, c0, ex)
                pump()
                flush(att_t, acc, ni, pend)
                # denominator: ones128 matmul partition-sums acc AND
                # broadcasts to all partitions; recip = exp(-ln x)
                recips = []
                for hh in range(2):
                    pump()
                    bc = pq.tile([128, 512], F32, tag="sc", name=f"bc{j}_{pr}_{hh}")
                    nc.tensor.matmul(
                        bc[:], ones_sb[:], acc[:, hh, :], start=True, stop=True
                    )
                    lnr = ps.tile(
                        [128, 512], F32, tag="lnr", name=f"lnr{j}_{pr}_{hh}", bufs=2
                    )
                    nc.scalar.activation(lnr[:], bc[:], AF.Ln)
                    recip = ps.tile(
                        [128, 512], F32, tag="recip", name=f"rc{j}_{pr}_{hh}", bufs=2
                    )
                    nc.scalar.activation(recip[:], lnr[:], AF.Exp, scale=-1.0)
                    recips.append(recip)
                pump()
                for hh in range(2):
                    nc.vector.tensor_tensor(
                        aT[2 * pr + hh][:, jsl],
                        att_t[:, hh, :],
                        recips[hh][:],
                        ALU.mult,
                    )

            def flush(att_t, acc, ni, pend):
                i, c0, ex = pend
                for hh in range(2):
                    nc.tensor.matmul(
                        att_t[:, hh, c0:],
                        vsb[:, i * 128 : (i + 1) * 128],
                        ex[:, hh, c0:],
                        start=(i == 0),
                        stop=(i == ni - 1),
                    )
                if i > 0:
                    nc.vector.tensor_tensor(
                        acc[:, :, c0:], acc[:, :, c0:], ex[:, :, c0:], ALU.add
                    )

            # ================ startup: strip 0 projections ================
            psA0 = pq.tile([128, 3, 512], F32, tag="proj", name="psA0")
            psBq0 = pq.tile([128, 2, 512], F32, tag="att", name="psBq0")
            psBv0 = pq.tile([128, 512], F32, tag="sc", name="psBv0")
            sl0 = slice(0, 512)
            nc.sync.dma_start(wq_c[0][:], wq3[:, CO[0] : CO[0] + CS[0], :])
            for ci, csz in enumerate(CS):
                kcs = slice(CO[ci], CO[ci] + csz)
                ht = ph.tile([128, csz, 512], BF, tag=f"ht{csz}", name=f"ht0_{ci}")
                if ci == 0:
                    # startup-critical transfers split over the scalar +
                    # sync hardware queues; consts go via gpsimd
                    nc.scalar.dma_start(ht[:], hT3[:, kcs, sl0])
                    nc.scalar.dma_start(wk_c[0][:], wk3[:, kcs, :])
                    nc.sync.dma_start(wv_c[0][:], wv3[:, kcs, :])
                    nc.gpsimd.dma_start(cos_sb[:], cosT[:])
                    nc.gpsimd.dma_start(sin_sb[:], sinT[:])
                    nc.gpsimd.dma_start(id_sb[:], ident[:])
                    nc.gpsimd.dma_start(
                        mask_sb[:].rearrange("p h m -> p (h m)"), masksd[:]
                    )
                    nc.gpsimd.dma_start(ones_sb[:], onesd[:])
                elif ci in (1, 2):
                    nc.scalar.dma_start(ht[:], hT3[:, kcs, sl0])
                else:
                    nc.sync.dma_start(ht[:], hT3[:, kcs, sl0])
                if ci + 1 < len(CS):
                    nkcs = slice(CO[ci + 1], CO[ci + 1] + CS[ci + 1])
                    nc.sync.dma_start(wq_c[ci + 1][:], wq3[:, nkcs, :])
                    nc.sync.dma_start(wk_c[ci + 1][:], wk3[:, nkcs, :])
                    nc.sync.dma_start(wv_c[ci + 1][:], wv3[:, nkcs, :])
                if 3 <= ci <= 6:
                    # wo needed from segment 1 (~60us in); stream on scalar
                    k4 = ci - 3
                    nc.scalar.dma_start(wo_sb[:, k4, :], wo3[:, k4, :])
                for kk in range(csz):
                    kt_i = CO[ci] + kk
                    st = kt_i == 0
                    sp_ = kt_i == KT - 1
                    rhs = ht[:, kk, :]
                    for sidx, h in enumerate((0, 1)):
                        nc.tensor.matmul(
                            psA0[:, sidx, :],
                            wq_c[ci][:, kk, h * 128 : (h + 1) * 128],
                            rhs, start=st, stop=sp_,
                        )
                    nc.tensor.matmul(
                        psA0[:, 2, :], wk_c[ci][:, kk, :], rhs, start=st, stop=sp_
                    )
                    for sidx, h in enumerate((2, 3)):
                        nc.tensor.matmul(
                            psBq0[:, sidx, :],
                            wq_c[ci][:, kk, h * 128 : (h + 1) * 128],
                            rhs, start=st, stop=sp_,
                        )
                    nc.tensor.matmul(
                        psBv0[:], wv_c[ci][:, kk, :], rhs, start=st, stop=sp_
                    )
            rope_A(0, psA0[:, 0:2, :], psA0[:, 2, :])
            rope_B(0, psBq0[:, 0:2, :], psBv0[:])

            # ================ fused segments ================
            O_CAP = {0: 0, 1: 20, 2: 20, 3: 10**9}
            emitted_o = set()

            for j in range(NSTRIP):
                proj_q.clear()
                if j < NSTRIP - 1:
                    proj_q.extend(build_proj_units(j + 1))
                added = 0
                cap = O_CAP[j]
                for jj in range(j):  # all finished strips, oldest first
                    for stt in range(4):
                        for mt in range(8):
                            key = (jj, stt, mt)
                            if key not in emitted_o and added < cap:
                                emitted_o.add(key)
                                o_q.append(
                                    (4 * 512,
                                     (lambda j_=jj, s_=stt, m_=mt:
                                      o_unit(j_, s_, m_, o_tag_next())))
                                )
                                added += 1
                total_cyc = sum(c for c, _ in proj_q) + sum(c for c, _ in o_q)
                slots = 2 * (4 * j + 4) + 8
                pump_state["per_slot"] = total_cyc // slots
                pump_state["credit"] = 0
                if j == NSTRIP - 1:
                    pump_state["o_tags"] = ["sp", "proj"]
                attn_pair(j, 0)
                attn_pair(j, 1)
                # proj for strip j+1 must be fully emitted before segment
                # j+1's attention reads it
                while proj_q:
                    c, fn = proj_q.pop(0)
                    fn()

            # ================ tail: remaining o_proj ================
            pump_state["o_tags"] = ["sp", "proj", "att", "sc"]
            while o_q:
                c, fn = o_q.pop(0)
                fn()
            for jj in range(NSTRIP):
                for stt in range(4):
                    for mt in range(8):
                        key = (jj, stt, mt)
                        if key not in emitted_o:
                            emitted_o.add(key)
                            o_unit(jj, stt, mt, o_tag_next())
            assert len(emitted_o) == NSTRIP * 4 * 8
    return nc


_cached_nc = None


def _get_nc():
    global _cached_nc
    if _cached_nc is None:
        nc = bass.Bass()
        _emit(nc)
        _split_excess_waits(nc)
        _cached_nc = nc
    return _cached_nc


def _bf(x):
    return np.ascontiguousarray(x.astype(ml_dtypes.bfloat16))


def _host_inputs(hidden_states, Wq, Wk, Wv, Wo):
    h = np.asarray(hidden_states, dtype=np.float32).reshape(S, HIDDEN)
    hT = _bf(h.T)

    inv = 1.0 / (ROPE_BASE ** (np.arange(0, HEAD_DIM, 2, dtype=np.float32) / HEAD_DIM))
    t = np.arange(S, dtype=np.float32)
    fr = np.outer(t, inv)
    emb = np.concatenate([fr, fr], axis=-1)  # [S, 128]
    cosT = _bf(np.cos(emb).T)
    sinT_ = np.sin(emb).T.copy()
    sinT_[0:64, :] *= -1.0  # sign of rotate-half folded into sin
    sinT = _bf(sinT_)

    identity = _bf(np.eye(128, dtype=np.float32))

    p = np.arange(128)[:, None]
    f = np.arange(128)[None, :]
    tri = (f >= p).astype(np.float32)  # [sk, q] lower-triangular in q>=sk sense
    masks = _bf(np.tile(tri, (1, 2)))
    ones128 = _bf(np.ones((128, 128), dtype=np.float32))

    Wq = np.asarray(Wq, dtype=np.float32)
    Wk = np.asarray(Wk, dtype=np.float32)
    Wv = np.asarray(Wv, dtype=np.float32)
    Wo = np.asarray(Wo, dtype=np.float32)

    in_maps = []
    for c in range(N_CORES):
        qs = slice(c * QH * HEAD_DIM, (c + 1) * QH * HEAD_DIM)
        ks = slice(c * HEAD_DIM, (c + 1) * HEAD_DIM)
        in_maps.append(
            dict(
                hT=hT,
                wqT=_bf(Wq[qs, :].T),
                wkT=_bf(Wk[ks, :].T),
                wvT=_bf(Wv[ks, :].T),
                woT=_bf(Wo[:, qs].T),
                cosT=cosT,
                sinT=sinT,
                ident=identity,
                masks=masks,
                ones=ones128,
            )
        )
    return in_maps


def _run(inputs, trace=False, tmpdir=None):
    nc = _get_nc()
    in_maps = _host_inputs(**inputs)
    res = run_bass_kernel_spmd(
        nc, in_maps, list(range(N_CORES)), trace=trace, tmpdir=tmpdir
    )
    o = np.zeros((S, HIDDEN), dtype=np.float32)
    for c in range(N_CORES):
        o += np.asarray(res.results[c]["o"]).astype(np.float32)
    return o.reshape(1, S, HIDDEN), res


def kernel(**inputs):
    o, _ = _run(inputs, trace=False)
    return o
